# revision 1
# baseline (speedup 1.0000x reference)
"""BitNetLinear forward on 8 Trainium2 NeuronCores.

Reference math (fp32):
    w_scale = mean(|W|)                         # scalar
    qW      = sign(W) * (|W| > 0.5*w_scale)     # ternary {-1,0,1}
    i_scale = max(|x|) / 127                    # global scalar over all of x
    qx      = clip(round(x / i_scale), -128, 127)
    out     = (qx @ qW.T) * w_scale * i_scale + bias

Strategy:
  * Data-parallel: core i gets batch element i -> x shard [4096, 1024].
    Weight (1024x1024) replicated on every core.
  * Host pre-transposes each x shard to [K=1024, M=4096] and W to
    [K, N] so the contraction dim lands on SBUF partitions for both
    matmul operands (pure layout prep; all math runs on device).
  * Phase 1 (per core): stream x shard into SBUF (kept resident, read
    from HBM exactly once) while reducing the local max|x|; a PE
    transpose collapses the per-partition maxima to one scalar, which
    goes through a 4-byte AllGather across the 8 cores.  Weight load,
    w_scale, ternary quantization and the bias load are placed after
    the collective trigger so they overlap its ~50us latency.
  * Phase 2: quantize activations from SBUF to bf16 and matmul.
    Quantized activations are integers <=127 and weights are {-1,0,1}:
    bf16 operands with fp32 PSUM accumulation are bit-exact here,
    giving the full bf16 tensor-engine rate instead of the fp32 rate.
    A short burst of discarded warm-up matmuls during the collective
    bridge lifts the PE clock gate (HAM) before the real stream starts.
  * round-to-nearest-even via the fp32 magic-constant trick
    (v + 1.5*2^23 - 1.5*2^23), matching jnp.round exactly.
"""

import sys

import numpy as np

sys.path.insert(0, "/opt/trn_rl_repo")

from concourse import bacc, mybir, tile  # noqa: E402
from concourse.bass_utils import run_bass_kernel_spmd  # noqa: E402


def _shim_ntff_hook():
    """Make run_bass_kernel_spmd's trace path importable even when this
    image's antenv lacks axon_hooks (it would otherwise crash on import if
    BASS_TRACE is set in the environment).  The no-op hook makes tracing
    degrade gracefully; a test harness may pre-register a real hook by
    installing its own antenv.axon_hooks before importing this module."""
    import types

    try:
        import antenv
    except ImportError:
        return
    if "antenv.axon_hooks" in sys.modules:
        return
    mod = types.ModuleType("antenv.axon_hooks")
    state = {"hook": None}
    mod.set_axon_ntff_profile_hook = lambda h: state.__setitem__("hook", h)
    mod.get_axon_ntff_profile_hook = lambda: state["hook"]
    sys.modules["antenv.axon_hooks"] = mod
    antenv.axon_hooks = mod


_shim_ntff_hook()

F32 = mybir.dt.float32
BF16 = mybir.dt.bfloat16
X = mybir.AxisListType.X
ALU = mybir.AluOpType
IDENT = mybir.ActivationFunctionType.Identity

P = 128          # SBUF partitions
K = 1024         # in_features
N = 1024         # out_features
KT = K // P      # 8 contraction tiles
N_CORES = 8
MB_Q = 512       # matmul-side chunk, in tokens
ACT_CHUNK = 1024  # activation-quantize chunk, in tokens (2x MB_Q)
C_MAGIC = 12582912.0  # 1.5 * 2**23, round-to-nearest-even bias
N_WARMUP_MM = 12  # discarded matmuls that lift the HAM clock gate

LAST_RESULT = None  # BassKernelResults of the most recent run (test harness peeks)

_PROGRAM_CACHE = {}


def build_program(m_tokens: int):
    """Emit the SPMD Bass/Tile program for one core (m_tokens tokens/core)."""
    M = m_tokens
    assert M % ACT_CHUNK == 0
    nqb = M // ACT_CHUNK

    nc = bacc.Bacc(
        "TRN2",
        target_bir_lowering=False,
        debug=False,
        enable_asserts=True,
        num_devices=N_CORES,
    )
    xt = nc.dram_tensor("xt", [K, M], F32, kind="ExternalInput").ap()
    wt = nc.dram_tensor("wt", [K, N], F32, kind="ExternalInput").ap()
    bias_b = nc.dram_tensor("bias_b", [P, N], F32, kind="ExternalInput").ap()
    ident = nc.dram_tensor("ident", [P, P], F32, kind="ExternalInput").ap()
    ones_r = nc.dram_tensor("ones_r", [1, P], F32, kind="ExternalInput").ap()
    out = nc.dram_tensor("out", [M, N], F32, kind="ExternalOutput").ap()

    rg = [list(range(N_CORES))]

    with tile.TileContext(nc) as tc:
        with (
            tc.tile_pool(name="xres", bufs=1) as xpool,
            tc.tile_pool(name="qw", bufs=1) as qwpool,
            tc.tile_pool(name="scal", bufs=1) as spool,
            tc.tile_pool(name="pehelp", bufs=1) as hpool,
            tc.tile_pool(name="psum", bufs=4, space="PSUM") as ppool,
            tc.tile_pool(name="dram", bufs=1, space="DRAM") as dpool,
        ):
            # identity (for PE transpose) and ones row (for PE broadcast)
            ident_t = hpool.tile([P, P], F32, tag="ident", name="ident_sb")
            nc.sync.dma_start(ident_t[:], ident[:])
            ones_t = hpool.tile([1, P], F32, tag="ones", name="ones_sb")
            nc.sync.dma_start(ones_t[:], ones_r[:])

            # ========== phase 1: x -> SBUF, local max, 4B AllGather ========
            # Only the x stream and its max reduction gate the collective;
            # everything else comes after the trigger in program order.
            xts = []
            H = M // 2
            partials = spool.tile([P, 2 * KT], F32, tag="pmax", name="pmax")
            for k in range(KT):
                xk = xpool.tile([P, M], F32, tag=f"x{k}", name=f"x_sb{k}")
                xts.append(xk)
                # DMA and reduce in half-tiles so the tail reduce starts as
                # soon as the last 1MB lands (not the last 2MB)
                for h in range(2):
                    nc.sync.dma_start(
                        xk[:, h * H : (h + 1) * H],
                        xt[k * P : (k + 1) * P, h * H : (h + 1) * H],
                    )
                    nc.vector.reduce_max(
                        partials[:, 2 * k + h : 2 * k + h + 1],
                        xk[:, h * H : (h + 1) * H],
                        axis=X,
                        apply_absolute_value=True,
                    )
            lmax = spool.tile([P, 1], F32, tag="lmax", name="lmax")
            nc.vector.reduce_max(lmax[:], partials[:], axis=X)

            # cross-partition reduce via PE transpose: [128,1] -> [1,128]
            # (scratch PSUM borrows rotating slots from the main matmul pool)
            tp = ppool.tile([1, P], F32, tag="ps", name="tp_ps")
            nc.tensor.transpose(tp[:], lmax[:], ident_t[:])
            lm_s = spool.tile([1, 1], F32, tag="lm_s", name="lm_s")
            nc.vector.reduce_max(lm_s[:], tp[:], axis=X)

            # 4-byte AllGather of the 8 per-core scalars
            cc_in = dpool.tile([1, 1], F32, name="cc_in")
            cc_out = dpool.tile(
                [1, N_CORES], F32, addr_space="Shared", name="cc_out"
            )
            # scalar engine's DMA queue is idle here; sync is busy with W loads
            nc.scalar.dma_start(cc_in[:], lm_s[:])
            nc.gpsimd.collective_compute(
                "AllGather",
                ALU.bypass,
                replica_groups=rg,
                ins=[cc_in.opt()],
                outs=[cc_out.opt()],
            )

            # ============== weight chain (overlaps the collective) ==========
            cmagic = spool.tile([P, 1], F32, tag="cmagic", name="cmagic")
            nc.vector.memset(cmagic[:], C_MAGIC)

            wsums = []
            qwts = []
            with (
                tc.tile_pool(name="wstream", bufs=3) as wpool,
                tc.tile_pool(name="wq_tmp", bufs=2) as wtpool,
            ):
                for k in range(KT):
                    wk = wpool.tile([P, N], F32, tag="w", name=f"w_sb{k}")
                    nc.sync.dma_start(wk[:], wt[k * P : (k + 1) * P, :])
                    sk = spool.tile([P, 1], F32, tag=f"ws{k}", name=f"wsum{k}")
                    nc.vector.reduce_sum(
                        sk[:], wk[:], axis=X, apply_absolute_value=True
                    )
                    wsums.append(sk)
                wsum = spool.tile([P, 1], F32, tag="wsum", name="wsum")
                nc.vector.tensor_add(wsum[:], wsums[0][:], wsums[1][:])
                for k in range(2, KT):
                    nc.vector.tensor_add(wsum[:], wsum[:], wsums[k][:])

                # cross-partition sum, again via PE transpose
                wtp = ppool.tile([1, P], F32, tag="ps", name="wtp_ps")
                nc.tensor.transpose(wtp[:], wsum[:], ident_t[:])
                ws_s = spool.tile([1, 1], F32, tag="ws_s", name="ws_s")
                nc.vector.reduce_sum(ws_s[:], wtp[:], axis=X)
                # broadcast to all partitions: ones^T @ scalar
                wbc = ppool.tile([P, 1], F32, tag="ps", name="wbc_ps")
                nc.tensor.matmul(
                    wbc[:], lhsT=ones_t[:], rhs=ws_s[:], start=True, stop=True
                )
                ws = spool.tile([P, 1], F32, tag="ws", name="ws")
                nc.vector.tensor_scalar_mul(ws[:], wbc[:], 1.0 / (K * N))
                inv_ws = spool.tile([P, 1], F32, tag="inv_ws", name="inv_ws")
                nc.vector.reciprocal(inv_ws[:], ws[:])
                ws127 = spool.tile([P, 1], F32, tag="ws127", name="ws127")
                nc.vector.tensor_scalar_mul(ws127[:], ws[:], 1.0 / 127.0)

                # ternary quantization:
                # qW = clip(round(W/ws), -1, 1)  (== sign(W)*(|W|>0.5*ws))
                for k in range(KT):
                    wk2 = wpool.tile([P, N], F32, tag="w", name=f"wq_sb{k}")
                    nc.sync.dma_start(wk2[:], wt[k * P : (k + 1) * P, :])
                    tq = wtpool.tile([P, N], F32, tag="t", name=f"wq_tmp{k}")
                    nc.scalar.activation(
                        tq[:], wk2[:], IDENT, bias=cmagic[:], scale=inv_ws[:]
                    )
                    qk = qwpool.tile([P, N], BF16, tag=f"qw{k}", name=f"qw_sb{k}")
                    nc.vector.tensor_scalar(
                        qk[:], tq[:], -C_MAGIC, 1.0, op0=ALU.add, op1=ALU.min
                    )
                    nc.vector.tensor_scalar_max(qk[:], qk[:], -1.0)
                    qwts.append(qk)

            with tc.tile_pool(name="biasp", bufs=1) as bpool:
                bias_t = bpool.tile([P, N], F32, tag="bias", name="bias_sb")
                nc.sync.dma_start(bias_t[:], bias_b[:])

                # ============== post-collective bridge ======================
                gm8 = spool.tile([1, N_CORES], F32, tag="gm8", name="gm8")
                nc.sync.dma_start(gm8[:], cc_out[:])
                g_s = spool.tile([1, 1], F32, tag="g_s", name="g_s")
                nc.vector.reduce_max(g_s[:], gm8[:], axis=X)
                gbc = ppool.tile([P, 1], F32, tag="ps", name="gbc_ps")
                nc.tensor.matmul(
                    gbc[:], lhsT=ones_t[:], rhs=g_s[:], start=True, stop=True
                )
                gmax = spool.tile([P, 1], F32, tag="gmax", name="gmax")
                nc.vector.tensor_copy(gmax[:], gbc[:])
                rcp = spool.tile([P, 1], F32, tag="rcp", name="rcp")
                nc.vector.reciprocal(rcp[:], gmax[:])
                inv_s = spool.tile([P, 1], F32, tag="inv_s", name="inv_s")
                nc.vector.tensor_scalar_mul(inv_s[:], rcp[:], 127.0)
                # sc = w_scale * i_scale = gmax * (ws/127)
                sc = spool.tile([P, 1], F32, tag="sc", name="sc")
                nc.vector.tensor_tensor(sc[:], gmax[:], ws127[:], op=ALU.mult)

                # PE warm-up: discarded matmuls right after the broadcast
                # matmul (which itself waits on the collective readback) keep
                # the PE busy through the bridge so HAM reaches 2.4GHz before
                # the real stream starts.  start=True groups; results are
                # funneled to a DRAM write so DCE keeps them.
                warm = ppool.tile([P, 512], F32, tag="ps", name="warm_ps")
                for j in range(N_WARMUP_MM):
                    nc.tensor.matmul(
                        warm[:],
                        lhsT=qwts[0][:, 0:P],
                        rhs=qwts[1][:, 0:512],
                        start=True,
                        stop=True,
                    )
                warm_sb = spool.tile([1, 1], F32, tag="warm_sb", name="warm_sb")
                nc.vector.tensor_copy(warm_sb[:], warm[0:1, 0:1])
                warm_dram = dpool.tile([1, 1], F32, name="warm_dram")
                nc.sync.dma_start(warm_dram[:], warm_sb[:])

                # ============== phase 2: quantize + matmul + dequant ========
                with (
                    tc.tile_pool(name="qstage", bufs=2) as qpool,
                    tc.tile_pool(name="qtmp", bufs=2) as tpool,
                    tc.tile_pool(name="ostage", bufs=3) as opool,
                ):
                    for qb in range(nqb):
                        m0 = qb * ACT_CHUNK
                        # quantize ACT_CHUNK tokens for all k-tiles:
                        # one wide ACT op, two half-chunk DVE casts to bf16
                        qs = []  # qs[k][half]
                        for k in range(KT):
                            tq = tpool.tile(
                                [P, ACT_CHUNK], F32, tag="tq", name=f"qtmp_{qb}_{k}"
                            )
                            nc.scalar.activation(
                                tq[:],
                                xts[k][:, m0 : m0 + ACT_CHUNK],
                                IDENT,
                                bias=cmagic[:],
                                scale=inv_s[:],
                            )
                            halves = []
                            for h in range(ACT_CHUNK // MB_Q):
                                qk = qpool.tile(
                                    [P, MB_Q], BF16, tag=f"q{k}_{h}",
                                    name=f"q_{qb}_{k}_{h}",
                                )
                                nc.vector.tensor_scalar_add(
                                    qk[:], tq[:, h * MB_Q : (h + 1) * MB_Q], -C_MAGIC
                                )
                                halves.append(qk)
                            qs.append(halves)
                        for mt in range(ACT_CHUNK // P):  # 8 m-tiles of 128
                            h, r = divmod(mt * P, MB_Q)
                            ps = ppool.tile(
                                [P, N], F32, tag="ps", name=f"ps_{qb}_{mt}"
                            )
                            for k in range(KT):
                                lhsT = qs[k][h][:, r : r + P]
                                for nh in range(2):
                                    mm = nc.tensor.matmul(
                                        ps[:, nh * 512 : (nh + 1) * 512],
                                        lhsT=lhsT,
                                        rhs=qwts[k][:, nh * 512 : (nh + 1) * 512],
                                        start=(k == 0),
                                        stop=(k == KT - 1),
                                    )
                                    if nh == 1:
                                        # same stationary as nh=0 — skip the
                                        # redundant weight load
                                        mm.ins.ldweights = False
                            ot = opool.tile([P, N], F32, tag="o", name=f"o_{qb}_{mt}")
                            nc.vector.scalar_tensor_tensor(
                                ot[:], ps[:], sc[:], bias_t[:],
                                op0=ALU.mult, op1=ALU.add,
                            )
                            row = m0 + mt * P
                            nc.sync.dma_start(out[row : row + P, :], ot[:])

    nc.compile()
    return nc


def _get_program(m_tokens: int):
    if m_tokens not in _PROGRAM_CACHE:
        _PROGRAM_CACHE[m_tokens] = build_program(m_tokens)
    return _PROGRAM_CACHE[m_tokens]


def kernel(x, weight, bias, **run_kwargs):
    """Full inputs in, full output out.  x:[8,4096,1024] w:[1024,1024] b:[1024]."""
    global LAST_RESULT
    x = np.asarray(x, dtype=np.float32)
    weight = np.asarray(weight, dtype=np.float32)
    bias = np.asarray(bias, dtype=np.float32)
    B, S, _K = x.shape
    assert B == N_CORES and _K == K

    # Host-side layout prep (sharding): feature-major shards + replicated W^T.
    xt_all = np.ascontiguousarray(x.transpose(0, 2, 1))        # [8, K, S]
    wt_host = np.ascontiguousarray(weight.T)                   # [K, N]
    bias_host = np.ascontiguousarray(
        np.broadcast_to(bias[None, :], (P, N))
    )                                                          # [P, N]
    ident_host = np.eye(P, dtype=np.float32)
    ones_host = np.ones((1, P), dtype=np.float32)

    nc = _get_program(S)
    in_maps = [
        {
            "xt": xt_all[i],
            "wt": wt_host,
            "bias_b": bias_host,
            "ident": ident_host,
            "ones_r": ones_host,
        }
        for i in range(N_CORES)
    ]
    res = run_bass_kernel_spmd(nc, in_maps, list(range(N_CORES)), **run_kwargs)
    LAST_RESULT = res
    return np.stack([res.results[i]["out"] for i in range(N_CORES)], axis=0)


if __name__ == "__main__":
    prog = build_program(4096)
    print("program built ok")

